# revision 25
# baseline (speedup 1.0000x reference)
"""Trainium2 Bass kernel for CodecAttention (GQA + full-width RMSNorm + ALiBi
+ 512 sliding causal window), SPMD over 8 NeuronCores.

Sharding: 2 batches x 4 sequence chunks of 512 queries per core. Each core
sees a [q0-512, q0+512) feature slice (zero-padded left halo for chunk 0),
computes its own QKV projections + norms + banded windowed attention + output
projection. Host only slices/transposes inputs and concatenates outputs.

v3 structure (vs v2 baseline):
- All matmul operands are bf16 (tolerance 2e-2 leaves >10x margin); halves
  DMA bytes and removes the f32r small-free-dim penalty.
- GQA head fusion: the 4 Q heads sharing a KV head are computed in single
  scores/PV matmuls (heads concatenated on the moving free dim), so one
  LDWEIGHTS covers 4 heads and every matmul moves >=512 rows.
- Attention loops q-chunk (128) outer, k-tile inner; every PSUM tile is 1-2
  banks so scores double-buffer + PV + bc/out fit in 8 banks.
- PV output lands at partitions 0:65 for even KV groups and 63:128 for odd
  ones (stationary [V|ones] vs [ones|V]), so attn head pairs assemble with
  no SBUF->SBUF partition-shift DMA.
- ALiBi via aug rows as in v2: scores matmul contraction carries (1.0,
  ramp-hi, ramp-lo) on the K side x (-8*slope*q, slope, slope) on the Q
  side; exp is then bias-free. Triangle masks post-exp via one 4-head
  affine_select per edge tile.
- Bulk DMAs issue from the Pool (gpsimd) sequencer (cheap DGE config) to
  kill the v2 startup serialization on the sync engine.
"""

import math

import numpy as np
import ml_dtypes

import concourse.bass as bass
import concourse.tile as tile
from concourse import bacc, mybir

F32 = mybir.dt.float32
F32R = mybir.dt.float32r
BF16 = mybir.dt.bfloat16
AF = mybir.ActivationFunctionType
ALU = mybir.AluOpType

# Problem constants (hardcoded per spec nn_CodecAttention_34308198761010)
B, S, M = 2, 2048, 1024
H, KV, D = 16, 4, 64
WIN = 512
SQ = 512          # queries per core
SK = 1024         # k-range per core (halo + chunk)
NCHUNK = S // SQ  # 4
N_CORES = 8
EPS = 1e-6
# Head order permutation (host-side, applied to wq/q_norm_w/wo): slot
# s = 2*t + par of wq tile t holds head PERM[s]; parity par matches the
# partition offset 64*par of its KV group g = 2*(t//4 pair) + par.
PERM = [0, 4, 1, 5, 2, 6, 3, 7, 8, 12, 9, 13, 10, 14, 11, 15]


def _alibi_slopes(n):
    ratio = 2.0 ** (-8.0 / n)
    return np.array([ratio**i for i in range(n)], dtype=np.float64)


def build_nc(for_sim=False):
    nc = bacc.Bacc(None, target_bir_lowering=False, debug=for_sim)

    feat_d = nc.dram_tensor("feat", [128, 8, SK], BF16, kind="ExternalInput")
    wq_d = nc.dram_tensor("wq", [128, 8, 8, 128], BF16, kind="ExternalInput")
    wk_d = nc.dram_tensor("wk", [128, 8, 256], BF16, kind="ExternalInput")
    wv_d = nc.dram_tensor("wv", [128, 8, 256], BF16, kind="ExternalInput")
    wo_d = nc.dram_tensor("wo", [128, 8, M], BF16, kind="ExternalInput")
    qw_d = nc.dram_tensor("qw", [128, 8], F32, kind="ExternalInput")
    kw_d = nc.dram_tensor("kw", [128, 2], F32, kind="ExternalInput")
    kaug_d = nc.dram_tensor("kaug", [3, SK], BF16, kind="ExternalInput")
    # qaug[par, r, gp, j, :]: aug rows r for slot (gp, par, j)
    qaug_d = nc.dram_tensor("qaug", [2, 3, 2, 4, SQ], BF16,
                            kind="ExternalInput")
    ones_d = nc.dram_tensor("onesin", [128, 1], F32R, kind="ExternalInput")
    # sel[:,0]: bcast partition 0; sel[:,1]: partition 64; sel[:,2]: 63
    sel_d = nc.dram_tensor("sel", [128, 3, 128], F32R, kind="ExternalInput")
    vone_d = nc.dram_tensor("vone", [128, 8, KV, 2], BF16,
                            kind="ExternalInput")
    out_d = nc.dram_tensor("out", [SQ, M], F32, kind="ExternalOutput")

    with tile.TileContext(nc) as tc:
        with (
            nc.allow_low_precision("bf16/f32r matmul operands intentional"),
            tc.tile_pool(name="const", bufs=1) as constp,
            tc.tile_pool(name="acts", bufs=1) as actsp,
            tc.tile_pool(name="sexp", bufs=3) as sexpp,
            tc.tile_pool(name="sqq", bufs=2) as sqqp,
            tc.tile_pool(name="small", bufs=1) as smallp,
            tc.tile_pool(name="dinv", bufs=4) as dinvp,
            tc.tile_pool(name="outsb", bufs=4) as outp,
        ):
            # ---- long-lived activations & constants ----
            # kt[:, g]: K data at partitions 64*(g%2)..+64, aug rows on the
            # opposite half (64:67 for even g, 0:3 for odd), zeros elsewhere.
            kt_sb = actsp.tile([128, KV, SK], BF16)
            # qt[:, gp, par, j]: head PERM[2*(4*gp+j)+par] data at
            # partitions 64*par..+64, aug rows opposite side, zeros rest.
            qt_sb = actsp.tile([128, 2, 2, 4, SQ], BF16)
            v_sb = actsp.tile([128, 8, KV, 66], BF16)
            attn_sb = actsp.tile([128, 8, SQ], BF16)
            wo_sb = actsp.tile([128, 8, M], BF16)

            qw_sb = constp.tile([128, 8], F32)
            kw_sb = constp.tile([128, 2], F32)
            ones_sb = constp.tile([128, 1], F32R)
            sel_sb = constp.tile([128, 3, 128], F32R)
            eps_sb = constp.tile([128, 1], F32)
            nc.vector.memset(eps_sb[:], EPS)
            nc.sync.dma_start(qw_sb[:], qw_d[:])
            nc.sync.dma_start(kw_sb[:], kw_d[:])

            # ================= projection phase (scoped SBUF) ==========
            with (
                tc.tile_pool(name="feat", bufs=1) as featp,
                tc.tile_pool(name="wkv", bufs=1) as wkvp,
                tc.tile_pool(name="wqpool", bufs=1) as wqp,
                tc.tile_pool(name="sqk", bufs=1) as sqkp,
            ):
                feat_sb = [featp.tile([128, SK], BF16, name=f"featc{mt}",
                                      tag=f"feat{mt}") for mt in range(8)]
                wk_sb = [wkvp.tile([128, 256], BF16, name=f"wkc{mt}",
                                   tag=f"wk{mt}") for mt in range(8)]
                wv_sb = [wkvp.tile([128, 256], BF16, name=f"wvc{mt}",
                                   tag=f"wv{mt}") for mt in range(8)]
                wq_sb = [wqp.tile([128, 8, 128], BF16, name=f"wqc{t}",
                                  tag=f"wq{t}") for t in range(8)]
                # gpsimd queue: feat/wk/wv first (needed earliest), then
                # memsets + the aug/vone loads that depend on them (same
                # in-order queue guarantees memset-before-aug)
                for mt in range(8):
                    nsl = 4 if mt == 0 else 2
                    for p in range(nsl):
                        w = 128 // nsl
                        ps = slice(w * p, w * p + w)
                        nc.gpsimd.dma_start(feat_sb[mt][ps, :],
                                            feat_d[ps, mt, :])
                    nc.gpsimd.dma_start(wk_sb[mt][:], wk_d[:, mt, :])
                    nc.gpsimd.dma_start(wv_sb[mt][:], wv_d[:, mt, :])
                nc.gpsimd.memset(kt_sb[:], 0.0)
                nc.gpsimd.memset(qt_sb[:], 0.0)
                for g in range(KV):
                    a0 = 64 * (1 - g % 2)
                    nc.gpsimd.dma_start(kt_sb[a0:a0 + 3, g, :], kaug_d[:, :])
                nc.gpsimd.dma_start(v_sb[:, :, :, 0:1], vone_d[:, :, :, 0:1])
                nc.gpsimd.dma_start(v_sb[:, :, :, 65:66],
                                    vone_d[:, :, :, 1:2])
                # sync queue: wq (needed ~12us) then wo (~60us) then qaug
                for t in range(8):
                    for p in range(2):
                        ps = slice(64 * p, 64 * p + 64)
                        nc.sync.dma_start(wq_sb[t][ps, :, :],
                                          wq_d[ps, t, :, :])
                nc.sync.dma_start(ones_sb[:], ones_d[:])
                nc.sync.dma_start(sel_sb[:], sel_d[:])
                for c in range(8):
                    for p in range(2):
                        ps = slice(64 * p, 64 * p + 64)
                        nc.sync.dma_start(wo_sb[ps, c, :], wo_d[ps, c, :])
                for par in range(2):
                    a0 = 64 * (1 - par)
                    nc.sync.dma_start(qt_sb[a0:a0 + 3, :, par, :, :],
                                      qaug_d[par])

                # ---- K+V projection, m-outer streaming ----
                with tc.tile_pool(name="psKV", bufs=1,
                                  space=bass.MemorySpace.PSUM) as psKV:
                    kp = psKV.tile([128, 2, SK], F32)
                    vp = psKV.tile([128, 4, 512], F32)
                    for mt in range(8):
                        for t in range(2):
                            for half in range(2):
                                cs = slice(512 * half, 512 * half + 512)
                                nc.tensor.matmul(
                                    kp[:, t, cs],
                                    wk_sb[mt][:, 128 * t:128 * t + 128],
                                    feat_sb[mt][:, cs],
                                    start=(mt == 0), stop=(mt == 7))
                        for st in range(4):
                            nc.tensor.matmul(
                                vp[:, st, 0:256],
                                feat_sb[mt][:, 128 * st:128 * st + 128],
                                wv_sb[mt][:],
                                start=(mt == 0), stop=(mt == 7))
                    for st in range(4):
                        nc.vector.tensor_copy(
                            v_sb[:, st, :, 1:65],
                            vp[:, st, 0:256].rearrange("p (g d) -> p g d",
                                                       g=KV))
                    vp2 = psKV.tile([128, 4, 512], F32, tag="vp")
                    for mt in range(8):
                        for st in range(4):
                            nc.tensor.matmul(
                                vp2[:, st, 0:256],
                                feat_sb[mt][:, 128 * (st + 4):128 * (st + 4) + 128],
                                wv_sb[mt][:],
                                start=(mt == 0), stop=(mt == 7))
                    for st in range(4):
                        nc.vector.tensor_copy(
                            v_sb[:, st + 4, :, 1:65],
                            vp2[:, st, 0:256].rearrange("p (g d) -> p g d",
                                                        g=KV))
                    # K squares (for rmsnorm) + raw K into kt slots with
                    # k_norm_w folded via Act per-partition scale
                    sqk = sqkp.tile([128, 2, SK], F32R)
                    nc.scalar.activation(sqk[:], kp[:], AF.Square)
                    for g in range(KV):
                        r = slice(64 * (g % 2), 64 * (g % 2) + 64)
                        t = g // 2
                        nc.scalar.activation(kt_sb[r, g, :], kp[r, t, :],
                                             AF.Copy,
                                             scale=kw_sb[r, t:t + 1])

                with (
                    tc.tile_pool(name="psPost", bufs=1,
                                 space=bass.MemorySpace.PSUM) as psPost,
                    tc.tile_pool(name="psQ", bufs=2,
                                 space=bass.MemorySpace.PSUM) as psQ,
                    tc.tile_pool(name="psSQ", bufs=1,
                                 space=bass.MemorySpace.PSUM) as psSQ,
                    tc.tile_pool(name="psBQ", bufs=1,
                                 space=bass.MemorySpace.PSUM) as psBQ,
                ):
                    # K rmsnorm: reduce sum-of-squares over the 256 k-dims,
                    # sqrt, broadcast, reciprocal, scale kt in place.
                    ssqk = psPost.tile([1, SK], F32)
                    for half in range(2):
                        cs = slice(512 * half, 512 * half + 512)
                        for t in range(2):
                            nc.tensor.matmul(ssqk[0:1, cs], ones_sb[:, 0:1],
                                             sqk[:, t, cs],
                                             start=(t == 0), stop=(t == 1))
                    srtk = smallp.tile([128, SK], F32R, tag="srtk")
                    nc.scalar.activation(srtk[0:1, :], ssqk[:], AF.Sqrt,
                                         scale=1.0 / 256.0,
                                         bias=eps_sb[0:1, :])
                    bck = psPost.tile([128, SK], F32)
                    for half in range(2):
                        cs = slice(512 * half, 512 * half + 512)
                        nc.tensor.matmul(bck[:, cs], sel_sb[:, 0, :],
                                         srtk[:, cs], start=True, stop=True)
                    bck_sb = smallp.tile([128, SK], F32, tag="bck")
                    nc.vector.reciprocal_approx_fast(bck_sb[:], bck[:])
                    for g in range(KV):
                        r = slice(64 * (g % 2), 64 * (g % 2) + 64)
                        nc.vector.tensor_mul(kt_sb[r, g, :], kt_sb[r, g, :],
                                             bck_sb[r, :])

                    # ---- Q projection + rmsnorm ----
                    ssqq = psSQ.tile([1, SQ], F32)
                    for t in range(8):
                        qp = psQ.tile([128, SQ], F32)
                        for mt in range(8):
                            nc.tensor.matmul(qp[:], wq_sb[t][:, mt, :],
                                             feat_sb[mt][:, 512:1024],
                                             start=(mt == 0), stop=(mt == 7))
                        sqq = sqqp.tile([128, SQ], F32R, tag="sqq")
                        nc.scalar.activation(sqq[:], qp[:], AF.Square)
                        nc.tensor.matmul(ssqq[:], ones_sb[:, 0:1], sqq[:],
                                         start=(t == 0), stop=(t == 7))
                        # data halves only (aug rows + zero fill untouched):
                        # par0 slot gets rows 0:64, par1 slot rows 64:128,
                        # q_norm_w folded in; split across DVE and Act
                        # (gpsimd cannot read PSUM)
                        gp, j = t // 4, t % 4
                        nc.vector.tensor_scalar_mul(
                            qt_sb[0:64, gp, 0, j, :], qp[0:64, :],
                            qw_sb[0:64, t:t + 1])
                        nc.scalar.activation(
                            qt_sb[64:128, gp, 1, j, :], qp[64:128, :],
                            AF.Copy, scale=qw_sb[64:128, t:t + 1])
                    srtq = smallp.tile([128, SQ], F32R, tag="srtq")
                    nc.scalar.activation(srtq[0:1, :], ssqq[:], AF.Sqrt,
                                         scale=1.0 / 1024.0,
                                         bias=eps_sb[0:1, :])
                    bcq = psBQ.tile([128, SQ], F32)
                    nc.tensor.matmul(bcq[:], sel_sb[:, 0, :], srtq[:, :],
                                     start=True, stop=True)
                    bcq_sb = smallp.tile([128, SQ], F32, tag="bcq")
                    nc.vector.reciprocal_approx_fast(bcq_sb[:], bcq[:])
                    for t in range(8):
                        gp, j = t // 4, t % 4
                        for par in range(2):
                            r = slice(64 * par, 64 * par + 64)
                            eng = nc.vector if par == 0 else nc.gpsimd
                            eng.tensor_mul(qt_sb[r, gp, par, j, :],
                                           qt_sb[r, gp, par, j, :],
                                           bcq_sb[r, :])

            # ================= attention phase ========================
            # per (q-chunk qc, KV group g): 5 k-tiles qc..qc+4; scores for
            # all 4 Q heads fused on the moving free dim; exp bias-free;
            # PV accumulates [V|ones] into a 1-bank PSUM tile.
            with (
                tc.tile_pool(name="psS", bufs=2,
                             space=bass.MemorySpace.PSUM) as psS,
                tc.tile_pool(name="psPV", bufs=2,
                             space=bass.MemorySpace.PSUM) as psPV,
                tc.tile_pool(name="psBO", bufs=2,
                             space=bass.MemorySpace.PSUM) as psBO,
            ):
                od = out_d.rearrange("(st p) m -> st p m", p=128)
                for qc in range(4):
                    qcs = slice(128 * qc, 128 * qc + 128)
                    for g in range(KV):
                        gp, par = g // 2, g % 2
                        rd = slice(64 * par, 64 * par + 64)  # data rows
                        qmov = qt_sb[:, gp, par, :, qcs]     # [128, 4, 128]
                        # kt order: unmasked first so masks stay off the
                        # critical path; (a)=(qc+1,qc+2), (b)=(qc+3,qc
                        # window-masked), (c)=(qc+4 causal-masked)
                        kts = [qc + 1, qc + 2, qc + 3, qc, qc + 4]
                        ses = []
                        for i in range(0, 5, 2):
                            pair = kts[i:i + 2]
                            sp = psS.tile([128, 2, 4, 128], F32,
                                          tag="scores")
                            for jj, kt in enumerate(pair):
                                nc.tensor.matmul(
                                    sp[:, jj],
                                    kt_sb[:, g, 128 * kt:128 * kt + 128],
                                    qmov, start=True, stop=True)
                            se = sexpp.tile([128, 2, 4, 128], BF16,
                                            tag="sexp")
                            np_ = len(pair)
                            nc.scalar.activation(se[:, 0:np_], sp[:, 0:np_],
                                                 AF.Exp, scale=0.125)
                            ses.append(se)
                        # window mask on kt=qc (keep k >= q-512: p-i >= 0)
                        nc.gpsimd.affine_select(
                            ses[1][:, 1], ses[1][:, 1],
                            pattern=[[0, 4], [-1, 128]],
                            compare_op=ALU.is_ge,
                            fill=0.0, base=0, channel_multiplier=1)
                        # causal mask on kt=qc+4 (keep k <= q: i-p >= 0)
                        nc.gpsimd.affine_select(
                            ses[2][:, 0], ses[2][:, 0],
                            pattern=[[0, 4], [1, 128]],
                            compare_op=ALU.is_ge,
                            fill=0.0, base=0, channel_multiplier=-1)
                        # PV: stationary [V|ones] -> out 0:65 (V dims at
                        # 0:64, denominator row at 64) for every group
                        pv = psPV.tile([128, 4, 128], F32, tag="pv")
                        for i, kt in enumerate(kts):
                            se = ses[i // 2][:, i % 2]
                            nc.tensor.matmul(
                                pv[0:65, :, :],
                                v_sb[:, kt, g, 1:66],
                                se, start=(i == 0), stop=(i == 4))
                        # normalize: attn = pv * bcast(1/denom). Copy the
                        # denom row to SBUF, broadcast via matmul, invert
                        # on DVE (so the final mul reads one PSUM operand).
                        dsb = dinvp.tile([128, 4, 128], F32R, tag="den")
                        nc.vector.tensor_copy(dsb[64:65], pv[64:65])
                        bc = psBO.tile([128, 4, 128], F32, tag="bco")
                        nc.tensor.matmul(bc[0:64], sel_sb[:, 1, 0:64],
                                         dsb[:], start=True, stop=True)
                        bci = dinvp.tile([128, 4, 128], F32, tag="bci")
                        nc.vector.reciprocal_approx_fast(bci[0:64], bc[0:64])
                        if par == 0:
                            nc.vector.tensor_mul(
                                attn_sb[0:64, 4 * gp:4 * gp + 4, qcs],
                                pv[0:64], bci[0:64])
                        else:
                            # odd-parity heads sit at attn rows 64:128;
                            # partition shift via SBUF->SBUF DMA (sync
                            # queue is idle during the attention phase)
                            atmp = dinvp.tile([64, 4, 128], BF16,
                                              tag="atmp")
                            nc.vector.tensor_mul(atmp[:], pv[0:64],
                                                 bci[0:64])
                            for p in range(2):
                                ps = slice(32 * p, 32 * p + 32)
                                pd = slice(64 + 32 * p, 64 + 32 * p + 32)
                                nc.sync.dma_start(
                                    attn_sb[pd, 4 * gp:4 * gp + 4, qcs],
                                    atmp[ps])
                    # ---- output projection for this q-chunk ----
                    osb = outp.tile([128, M], F32, tag="osb")
                    for mh in range(2):
                        op = psBO.tile([128, 4, 128], F32, tag="bco")
                        opf = op.rearrange("p a b -> p (a b)")
                        for ht in range(8):
                            nc.tensor.matmul(
                                opf[:],
                                attn_sb[:, ht, qcs],
                                wo_sb[:, ht, 512 * mh:512 * mh + 512],
                                start=(ht == 0), stop=(ht == 7))
                        nc.vector.tensor_copy(
                            osb[:, 512 * mh:512 * mh + 512], opf[:])
                    for p in range(8):
                        ps = slice(16 * p, 16 * p + 16)
                        nc.gpsimd.dma_start(od[qc][ps, :], osb[ps, :])

    if for_sim:
        nc.compile()
    else:
        nc.finalize()
    return nc


def make_in_maps(features, wq, wk, wv, wo, q_norm_w, k_norm_w):
    bf16 = ml_dtypes.bfloat16
    features = np.asarray(features, np.float32)
    wq = np.asarray(wq, np.float32)
    wk = np.asarray(wk, np.float32)
    wv = np.asarray(wv, np.float32)
    wo = np.asarray(wo, np.float32)
    q_norm_w = np.asarray(q_norm_w, np.float32)
    k_norm_w = np.asarray(k_norm_w, np.float32)

    # permute Q-head order (see PERM) in wq rows, q_norm_w, wo columns
    wq_p = wq.reshape(H, D, M)[PERM].reshape(H * D, M)
    qnw_p = q_norm_w.reshape(H, D)[PERM].reshape(H * D)
    wo_tp = wo.T.reshape(H, D, M)[PERM].reshape(H * D, M)  # wo.T rows = hd

    wq_pre = np.ascontiguousarray(
        wq_p.T.reshape(8, 128, 8, 128).transpose(1, 2, 0, 3)).astype(bf16)
    wk_pre = np.ascontiguousarray(
        wk.T.reshape(8, 128, 256).transpose(1, 0, 2)).astype(bf16)
    wv_pre = np.ascontiguousarray(
        wv.T.reshape(8, 128, 256).transpose(1, 0, 2)).astype(bf16)
    wo_pre = np.ascontiguousarray(
        wo_tp.reshape(8, 128, M).transpose(1, 0, 2)).astype(bf16)
    qw_pre = np.ascontiguousarray(qnw_p.reshape(8, 128).T)
    kw_pre = np.ascontiguousarray(k_norm_w.reshape(2, 128).T)

    slopes = _alibi_slopes(H)
    # K-side aug rows: row 0 = 1.0 (pairs with -8*slope*q), rows 1,2 carry
    # the ramp 8*(kpos-512) split into pieces exactly representable in
    # bf16's 8-bit significand: 8m = 32*(m//4) + 8*(m%4).
    m = np.arange(SK, dtype=np.int64) - 512
    kaug = np.zeros((3, SK), np.float32)
    kaug[0, :] = 1.0
    kaug[1, :] = 32.0 * (m // 4).astype(np.float64)
    kaug[2, :] = 8.0 * (m % 4).astype(np.float64)
    # Q-side aug rows per slot (par, gp, j) -> head PERM[2*(4*gp+j)+par].
    # Row 0 is a per-q constant, so its bf16 rounding cancels in softmax.
    qaug = np.zeros((2, 3, 2, 4, SQ), np.float32)
    qi = np.arange(SQ, dtype=np.float64)
    for par in range(2):
        for gp in range(2):
            for j in range(4):
                h = PERM[2 * (4 * gp + j) + par]
                qaug[par, 0, gp, j, :] = -8.0 * slopes[h] * qi
                qaug[par, 1, gp, j, :] = slopes[h]
                qaug[par, 2, gp, j, :] = slopes[h]
    # row selectors
    sel = np.zeros((128, 3, 128), np.float32)
    sel[0, 0, :] = 1.0
    sel[64, 1, :] = 1.0
    sel[63, 2, :] = 1.0

    in_maps = []
    for b in range(B):
        for c in range(NCHUNK):
            q0 = c * SQ
            lo, hi = q0 - WIN, q0 + SQ
            fs = np.zeros((SK, M), np.float32)
            src_lo = max(lo, 0)
            fs[src_lo - lo:, :] = features[b, src_lo:hi, :]
            feat_pre = np.ascontiguousarray(
                fs.T.reshape(8, 128, SK).transpose(1, 0, 2)).astype(bf16)
            # ones column of V doubles as the chunk-0 halo mask: a zero
            # kills both the PV contribution and the denominator term.
            vone = np.ones((128, 8, KV, 2), np.float32)
            if c == 0:
                vone[:, 0:4, :, :] = 0.0
            in_maps.append({
                "feat": feat_pre, "wq": wq_pre, "wk": wk_pre, "wv": wv_pre,
                "wo": wo_pre, "qw": qw_pre, "kw": kw_pre,
                "kaug": kaug.astype(bf16), "qaug": qaug.astype(bf16),
                "sel": sel, "onesin": np.ones((128, 1), np.float32),
                "vone": vone.astype(bf16),
            })
    return in_maps


_NC_CACHE = {}


def kernel(features, wq, wk, wv, wo, q_norm_w, k_norm_w,
           num_heads=16, num_kv_heads=4, head_dim=64, sliding_window=512,
           **_unused):
    assert int(num_heads) == H and int(num_kv_heads) == KV
    assert int(head_dim) == D and int(sliding_window) == WIN
    from concourse.bass_utils import run_bass_kernel_spmd

    if "nc" not in _NC_CACHE:
        _NC_CACHE["nc"] = build_nc(for_sim=False)
    nc = _NC_CACHE["nc"]
    in_maps = make_in_maps(features, wq, wk, wv, wo, q_norm_w, k_norm_w)
    res = run_bass_kernel_spmd(nc, in_maps, core_ids=list(range(N_CORES)))
    outs = [r["out"] for r in res.results]
    full = np.stack(outs, axis=0).reshape(B, NCHUNK * SQ, M)
    return full.astype(np.float32)


# revision 26
# speedup vs baseline: 1.0237x; 1.0237x over previous
"""Trainium2 Bass kernel for CodecAttention (GQA + full-width RMSNorm + ALiBi
+ 512 sliding causal window), SPMD over 8 NeuronCores.

Sharding: 2 batches x 4 sequence chunks of 512 queries per core. Each core
sees a [q0-512, q0+512) feature slice (zero-padded left halo for chunk 0),
computes its own QKV projections + norms + banded windowed attention + output
projection. Host only slices/transposes inputs and concatenates outputs.

v3 structure (vs v2 baseline):
- All matmul operands are bf16 (tolerance 2e-2 leaves >10x margin); halves
  DMA bytes and removes the f32r small-free-dim penalty.
- GQA head fusion: the 4 Q heads sharing a KV head are computed in single
  scores/PV matmuls (heads concatenated on the moving free dim), so one
  LDWEIGHTS covers 4 heads and every matmul moves >=512 rows.
- Attention loops q-chunk (128) outer, k-tile inner; every PSUM tile is 1-2
  banks so scores double-buffer + PV + bc/out fit in 8 banks.
- PV output lands at partitions 0:65 for even KV groups and 63:128 for odd
  ones (stationary [V|ones] vs [ones|V]), so attn head pairs assemble with
  no SBUF->SBUF partition-shift DMA.
- ALiBi via aug rows as in v2: scores matmul contraction carries (1.0,
  ramp-hi, ramp-lo) on the K side x (-8*slope*q, slope, slope) on the Q
  side; exp is then bias-free. Triangle masks post-exp via one 4-head
  affine_select per edge tile.
- Bulk DMAs issue from the Pool (gpsimd) sequencer (cheap DGE config) to
  kill the v2 startup serialization on the sync engine.
"""

import math

import numpy as np
import ml_dtypes

import concourse.bass as bass
import concourse.tile as tile
from concourse import bacc, mybir

F32 = mybir.dt.float32
F32R = mybir.dt.float32r
BF16 = mybir.dt.bfloat16
AF = mybir.ActivationFunctionType
ALU = mybir.AluOpType

# Problem constants (hardcoded per spec nn_CodecAttention_34308198761010)
B, S, M = 2, 2048, 1024
H, KV, D = 16, 4, 64
WIN = 512
SQ = 512          # queries per core
SK = 1024         # k-range per core (halo + chunk)
NCHUNK = S // SQ  # 4
N_CORES = 8
EPS = 1e-6
# Head order permutation (host-side, applied to wq/q_norm_w/wo): slot
# s = 2*t + par of wq tile t holds head PERM[s]; parity par matches the
# partition offset 64*par of its KV group g = 2*(t//4 pair) + par.
PERM = [0, 4, 1, 5, 2, 6, 3, 7, 8, 12, 9, 13, 10, 14, 11, 15]


def _alibi_slopes(n):
    ratio = 2.0 ** (-8.0 / n)
    return np.array([ratio**i for i in range(n)], dtype=np.float64)


def build_nc(for_sim=False):
    nc = bacc.Bacc(None, target_bir_lowering=False, debug=for_sim)

    feat_d = nc.dram_tensor("feat", [128, 8, SK], BF16, kind="ExternalInput")
    wq_d = nc.dram_tensor("wq", [128, 8, 8, 128], BF16, kind="ExternalInput")
    wk_d = nc.dram_tensor("wk", [128, 8, 256], BF16, kind="ExternalInput")
    wv_d = nc.dram_tensor("wv", [128, 8, 256], BF16, kind="ExternalInput")
    wo_d = nc.dram_tensor("wo", [128, 8, M], BF16, kind="ExternalInput")
    qw_d = nc.dram_tensor("qw", [128, 8], F32, kind="ExternalInput")
    kw_d = nc.dram_tensor("kw", [128, 2], F32, kind="ExternalInput")
    kaug_d = nc.dram_tensor("kaug", [3, SK], BF16, kind="ExternalInput")
    # qaug[par, r, gp, j, :]: aug rows r for slot (gp, par, j)
    qaug_d = nc.dram_tensor("qaug", [2, 3, 2, 4, SQ], BF16,
                            kind="ExternalInput")
    ones_d = nc.dram_tensor("onesin", [128, 1], F32R, kind="ExternalInput")
    # sel[:,0]: bcast partition 0; sel[:,1]: partition 64; sel[:,2]: 63
    sel_d = nc.dram_tensor("sel", [128, 3, 128], F32R, kind="ExternalInput")
    vone_d = nc.dram_tensor("vone", [128, 8, KV, 2], BF16,
                            kind="ExternalInput")
    out_d = nc.dram_tensor("out", [SQ, M], F32, kind="ExternalOutput")

    with tile.TileContext(nc) as tc:
        with (
            nc.allow_low_precision("bf16/f32r matmul operands intentional"),
            tc.tile_pool(name="const", bufs=1) as constp,
            tc.tile_pool(name="acts", bufs=1) as actsp,
            tc.tile_pool(name="sexp", bufs=3) as sexpp,
            tc.tile_pool(name="sqq", bufs=2) as sqqp,
            tc.tile_pool(name="small", bufs=1) as smallp,
            tc.tile_pool(name="dinv", bufs=4) as dinvp,
            tc.tile_pool(name="outsb", bufs=4) as outp,
        ):
            # ---- long-lived activations & constants ----
            # kt[:, g]: K data at partitions 64*(g%2)..+64, aug rows on the
            # opposite half (64:67 for even g, 0:3 for odd), zeros elsewhere.
            kt_sb = actsp.tile([128, KV, SK], BF16)
            # qt[:, gp, par, j]: head PERM[2*(4*gp+j)+par] data at
            # partitions 64*par..+64, aug rows opposite side, zeros rest.
            qt_sb = actsp.tile([128, 2, 2, 4, SQ], BF16)
            v_sb = actsp.tile([128, 8, KV, 66], BF16)
            attn_sb = actsp.tile([128, 8, SQ], BF16)
            wo_sb = actsp.tile([128, 8, M], BF16)

            qw_sb = constp.tile([128, 8], F32)
            kw_sb = constp.tile([128, 2], F32)
            ones_sb = constp.tile([128, 1], F32R)
            sel_sb = constp.tile([128, 3, 128], F32R)
            eps_sb = constp.tile([128, 1], F32)
            nc.vector.memset(eps_sb[:], EPS)
            nc.sync.dma_start(qw_sb[:], qw_d[:])
            nc.sync.dma_start(kw_sb[:], kw_d[:])

            # ================= projection phase (scoped SBUF) ==========
            with (
                tc.tile_pool(name="feat", bufs=1) as featp,
                tc.tile_pool(name="wkv", bufs=1) as wkvp,
                tc.tile_pool(name="wqpool", bufs=1) as wqp,
                tc.tile_pool(name="sqk", bufs=1) as sqkp,
            ):
                feat_sb = [featp.tile([128, SK], BF16, name=f"featc{mt}",
                                      tag=f"feat{mt}") for mt in range(8)]
                wk_sb = [wkvp.tile([128, 256], BF16, name=f"wkc{mt}",
                                   tag=f"wk{mt}") for mt in range(8)]
                wv_sb = [wkvp.tile([128, 256], BF16, name=f"wvc{mt}",
                                   tag=f"wv{mt}") for mt in range(8)]
                wq_sb = [wqp.tile([128, 8, 128], BF16, name=f"wqc{t}",
                                  tag=f"wq{t}") for t in range(8)]
                # gpsimd queue: feat/wk/wv first (needed earliest), then
                # memsets + the aug/vone loads that depend on them (same
                # in-order queue guarantees memset-before-aug)
                for mt in range(8):
                    nsl = 4 if mt == 0 else 2
                    for p in range(nsl):
                        w = 128 // nsl
                        ps = slice(w * p, w * p + w)
                        nc.gpsimd.dma_start(feat_sb[mt][ps, :],
                                            feat_d[ps, mt, :])
                    nc.gpsimd.dma_start(wk_sb[mt][:], wk_d[:, mt, :])
                    nc.gpsimd.dma_start(wv_sb[mt][:], wv_d[:, mt, :])
                nc.gpsimd.memset(kt_sb[:], 0.0)
                nc.gpsimd.memset(qt_sb[:], 0.0)
                for g in range(KV):
                    a0 = 64 * (1 - g % 2)
                    nc.gpsimd.dma_start(kt_sb[a0:a0 + 3, g, :], kaug_d[:, :])
                nc.gpsimd.dma_start(v_sb[:, :, :, 0:1], vone_d[:, :, :, 0:1])
                nc.gpsimd.dma_start(v_sb[:, :, :, 65:66],
                                    vone_d[:, :, :, 1:2])
                # sync queue: wq (needed ~12us) then wo (~60us) then qaug
                for t in range(8):
                    for p in range(2):
                        ps = slice(64 * p, 64 * p + 64)
                        nc.sync.dma_start(wq_sb[t][ps, :, :],
                                          wq_d[ps, t, :, :])
                nc.sync.dma_start(ones_sb[:], ones_d[:])
                nc.sync.dma_start(sel_sb[:], sel_d[:])
                for c in range(8):
                    for p in range(2):
                        ps = slice(64 * p, 64 * p + 64)
                        nc.sync.dma_start(wo_sb[ps, c, :], wo_d[ps, c, :])
                for par in range(2):
                    a0 = 64 * (1 - par)
                    nc.sync.dma_start(qt_sb[a0:a0 + 3, :, par, :, :],
                                      qaug_d[par])

                # ---- K+V projection, m-outer streaming ----
                with tc.tile_pool(name="psKV", bufs=1,
                                  space=bass.MemorySpace.PSUM) as psKV:
                    kp = psKV.tile([128, 2, SK], F32)
                    vp = psKV.tile([128, 4, 512], F32)
                    for mt in range(8):
                        for t in range(2):
                            for half in range(2):
                                cs = slice(512 * half, 512 * half + 512)
                                nc.tensor.matmul(
                                    kp[:, t, cs],
                                    wk_sb[mt][:, 128 * t:128 * t + 128],
                                    feat_sb[mt][:, cs],
                                    start=(mt == 0), stop=(mt == 7))
                        for st in range(4):
                            nc.tensor.matmul(
                                vp[:, st, 0:256],
                                feat_sb[mt][:, 128 * st:128 * st + 128],
                                wv_sb[mt][:],
                                start=(mt == 0), stop=(mt == 7))
                    for st in range(4):
                        nc.vector.tensor_copy(
                            v_sb[:, st, :, 1:65],
                            vp[:, st, 0:256].rearrange("p (g d) -> p g d",
                                                       g=KV))
                    vp2 = psKV.tile([128, 4, 512], F32, tag="vp")
                    for mt in range(8):
                        for st in range(4):
                            nc.tensor.matmul(
                                vp2[:, st, 0:256],
                                feat_sb[mt][:, 128 * (st + 4):128 * (st + 4) + 128],
                                wv_sb[mt][:],
                                start=(mt == 0), stop=(mt == 7))
                    for st in range(4):
                        nc.vector.tensor_copy(
                            v_sb[:, st + 4, :, 1:65],
                            vp2[:, st, 0:256].rearrange("p (g d) -> p g d",
                                                        g=KV))
                    # K squares (for rmsnorm) + raw K into kt slots with
                    # k_norm_w folded via Act per-partition scale
                    sqk = sqkp.tile([128, 2, SK], F32R)
                    nc.scalar.activation(sqk[:], kp[:], AF.Square)
                    for g in range(KV):
                        r = slice(64 * (g % 2), 64 * (g % 2) + 64)
                        t = g // 2
                        nc.scalar.activation(kt_sb[r, g, :], kp[r, t, :],
                                             AF.Copy,
                                             scale=kw_sb[r, t:t + 1])

                with (
                    tc.tile_pool(name="psPost", bufs=1,
                                 space=bass.MemorySpace.PSUM) as psPost,
                    tc.tile_pool(name="psQ", bufs=2,
                                 space=bass.MemorySpace.PSUM) as psQ,
                    tc.tile_pool(name="psSQ", bufs=1,
                                 space=bass.MemorySpace.PSUM) as psSQ,
                    tc.tile_pool(name="psBQ", bufs=1,
                                 space=bass.MemorySpace.PSUM) as psBQ,
                ):
                    # K rmsnorm: reduce sum-of-squares over the 256 k-dims,
                    # sqrt, broadcast, reciprocal, scale kt in place.
                    ssqk = psPost.tile([1, SK], F32)
                    for half in range(2):
                        cs = slice(512 * half, 512 * half + 512)
                        for t in range(2):
                            nc.tensor.matmul(ssqk[0:1, cs], ones_sb[:, 0:1],
                                             sqk[:, t, cs],
                                             start=(t == 0), stop=(t == 1))
                    srtk = smallp.tile([128, SK], F32R, tag="srtk")
                    nc.scalar.activation(srtk[0:1, :], ssqk[:], AF.Sqrt,
                                         scale=1.0 / 256.0,
                                         bias=eps_sb[0:1, :])
                    bck = psPost.tile([128, SK], F32)
                    for half in range(2):
                        cs = slice(512 * half, 512 * half + 512)
                        nc.tensor.matmul(bck[:, cs], sel_sb[:, 0, :],
                                         srtk[:, cs], start=True, stop=True)
                    bck_sb = smallp.tile([128, SK], F32, tag="bck")
                    nc.vector.reciprocal_approx_fast(bck_sb[:], bck[:])
                    for g in range(KV):
                        r = slice(64 * (g % 2), 64 * (g % 2) + 64)
                        nc.vector.tensor_mul(kt_sb[r, g, :], kt_sb[r, g, :],
                                             bck_sb[r, :])

                    # ---- Q projection + rmsnorm ----
                    ssqq = psSQ.tile([1, SQ], F32)
                    for t in range(8):
                        qp = psQ.tile([128, SQ], F32)
                        for mt in range(8):
                            nc.tensor.matmul(qp[:], wq_sb[t][:, mt, :],
                                             feat_sb[mt][:, 512:1024],
                                             start=(mt == 0), stop=(mt == 7))
                        sqq = sqqp.tile([128, SQ], F32R, tag="sqq")
                        nc.scalar.activation(sqq[:], qp[:], AF.Square)
                        nc.tensor.matmul(ssqq[:], ones_sb[:, 0:1], sqq[:],
                                         start=(t == 0), stop=(t == 7))
                        # data halves only (aug rows + zero fill untouched):
                        # par0 slot gets rows 0:64, par1 slot rows 64:128,
                        # q_norm_w folded in; split across DVE and Act
                        # (gpsimd cannot read PSUM)
                        gp, j = t // 4, t % 4
                        nc.vector.tensor_scalar_mul(
                            qt_sb[0:64, gp, 0, j, :], qp[0:64, :],
                            qw_sb[0:64, t:t + 1])
                        nc.scalar.activation(
                            qt_sb[64:128, gp, 1, j, :], qp[64:128, :],
                            AF.Copy, scale=qw_sb[64:128, t:t + 1])
                    srtq = smallp.tile([128, SQ], F32R, tag="srtq")
                    nc.scalar.activation(srtq[0:1, :], ssqq[:], AF.Sqrt,
                                         scale=1.0 / 1024.0,
                                         bias=eps_sb[0:1, :])
                    bcq = psBQ.tile([128, SQ], F32)
                    nc.tensor.matmul(bcq[:], sel_sb[:, 0, :], srtq[:, :],
                                     start=True, stop=True)
                    bcq_sb = smallp.tile([128, SQ], F32, tag="bcq")
                    nc.vector.reciprocal_approx_fast(bcq_sb[:], bcq[:])
                    for t in range(8):
                        gp, j = t // 4, t % 4
                        for par in range(2):
                            r = slice(64 * par, 64 * par + 64)
                            eng = nc.vector if par == 0 else nc.gpsimd
                            eng.tensor_mul(qt_sb[r, gp, par, j, :],
                                           qt_sb[r, gp, par, j, :],
                                           bcq_sb[r, :])

            # ================= attention phase ========================
            # per (q-chunk qc, KV group g): 5 k-tiles qc..qc+4; scores for
            # all 4 Q heads fused on the moving free dim; exp bias-free;
            # PV accumulates [V|ones] into a 1-bank PSUM tile.
            with (
                tc.tile_pool(name="psS", bufs=2,
                             space=bass.MemorySpace.PSUM) as psS,
                tc.tile_pool(name="psPV", bufs=2,
                             space=bass.MemorySpace.PSUM) as psPV,
                tc.tile_pool(name="psBO", bufs=2,
                             space=bass.MemorySpace.PSUM) as psBO,
            ):
                od = out_d.rearrange("(st p) m -> st p m", p=128)
                for qc in range(4):
                    qcs = slice(128 * qc, 128 * qc + 128)
                    for g in range(KV):
                        gp, par = g // 2, g % 2
                        rd = slice(64 * par, 64 * par + 64)  # data rows
                        qmov = qt_sb[:, gp, par, :, qcs]     # [128, 4, 128]
                        # kt order: unmasked first so masks stay off the
                        # critical path; (a)=(qc+1,qc+2), (b)=(qc+3,qc
                        # window-masked), (c)=(qc+4 causal-masked)
                        kts = [qc + 1, qc + 2, qc + 3, qc, qc + 4]
                        ses = []
                        for i in range(0, 5, 2):
                            pair = kts[i:i + 2]
                            sp = psS.tile([128, 2, 4, 128], F32,
                                          tag="scores")
                            for jj, kt in enumerate(pair):
                                nc.tensor.matmul(
                                    sp[:, jj],
                                    kt_sb[:, g, 128 * kt:128 * kt + 128],
                                    qmov, start=True, stop=True)
                            se = sexpp.tile([128, 2, 4, 128], BF16,
                                            tag="sexp")
                            np_ = len(pair)
                            nc.scalar.activation(se[:, 0:np_], sp[:, 0:np_],
                                                 AF.Exp, scale=0.125)
                            ses.append(se)
                        # window mask on kt=qc (keep k >= q-512: p-i >= 0)
                        nc.gpsimd.affine_select(
                            ses[1][:, 1], ses[1][:, 1],
                            pattern=[[0, 4], [-1, 128]],
                            compare_op=ALU.is_ge,
                            fill=0.0, base=0, channel_multiplier=1)
                        # causal mask on kt=qc+4 (keep k <= q: i-p >= 0)
                        nc.gpsimd.affine_select(
                            ses[2][:, 0], ses[2][:, 0],
                            pattern=[[0, 4], [1, 128]],
                            compare_op=ALU.is_ge,
                            fill=0.0, base=0, channel_multiplier=-1)
                        # PV: stationary [V|ones] -> out 0:65 (V dims at
                        # 0:64, denominator row at 64) for every group
                        pv = psPV.tile([128, 4, 128], F32, tag="pv")
                        for i, kt in enumerate(kts):
                            se = ses[i // 2][:, i % 2]
                            nc.tensor.matmul(
                                pv[0:65, :, :],
                                v_sb[:, kt, g, 1:66],
                                se, start=(i == 0), stop=(i == 4))
                        # normalize: attn = pv * bcast(1/denom). Copy the
                        # denom row to SBUF, broadcast via matmul, invert
                        # on DVE (so the final mul reads one PSUM operand).
                        dsb = dinvp.tile([128, 4, 128], F32R, tag="den")
                        nc.vector.tensor_copy(dsb[64:65], pv[64:65])
                        bc = psBO.tile([128, 4, 128], F32, tag="bco")
                        nc.tensor.matmul(bc[0:64], sel_sb[:, 1, 0:64],
                                         dsb[:], start=True, stop=True)
                        bci = dinvp.tile([128, 4, 128], F32, tag="bci")
                        nc.vector.reciprocal_approx_fast(bci[0:64], bc[0:64])
                        if par == 0:
                            nc.vector.tensor_mul(
                                attn_sb[0:64, 4 * gp:4 * gp + 4, qcs],
                                pv[0:64], bci[0:64])
                        else:
                            # odd-parity heads sit at attn rows 64:128;
                            # partition shift via SBUF->SBUF DMA (sync
                            # queue is idle during the attention phase)
                            atmp = dinvp.tile([64, 4, 128], BF16,
                                              tag="atmp")
                            nc.vector.tensor_mul(atmp[:], pv[0:64],
                                                 bci[0:64])
                            for p in range(2):
                                ps = slice(32 * p, 32 * p + 32)
                                pd = slice(64 + 32 * p, 64 + 32 * p + 32)
                                nc.sync.dma_start(
                                    attn_sb[pd, 4 * gp:4 * gp + 4, qcs],
                                    atmp[ps])
                    # ---- output projection for this q-chunk ----
                    osb = outp.tile([128, M], F32, tag="osb")
                    for mh in range(2):
                        op = psBO.tile([128, 4, 128], F32, tag="bco")
                        opf = op.rearrange("p a b -> p (a b)")
                        for ht in range(8):
                            nc.tensor.matmul(
                                opf[:],
                                attn_sb[:, ht, qcs],
                                wo_sb[:, ht, 512 * mh:512 * mh + 512],
                                start=(ht == 0), stop=(ht == 7))
                        nc.vector.tensor_copy(
                            osb[:, 512 * mh:512 * mh + 512], opf[:])
                    for p in range(8):
                        ps = slice(16 * p, 16 * p + 16)
                        nc.sync.dma_start(od[qc][ps, :], osb[ps, :])

    if for_sim:
        nc.compile()
    else:
        nc.finalize()
    return nc


def make_in_maps(features, wq, wk, wv, wo, q_norm_w, k_norm_w):
    bf16 = ml_dtypes.bfloat16
    features = np.asarray(features, np.float32)
    wq = np.asarray(wq, np.float32)
    wk = np.asarray(wk, np.float32)
    wv = np.asarray(wv, np.float32)
    wo = np.asarray(wo, np.float32)
    q_norm_w = np.asarray(q_norm_w, np.float32)
    k_norm_w = np.asarray(k_norm_w, np.float32)

    # permute Q-head order (see PERM) in wq rows, q_norm_w, wo columns
    wq_p = wq.reshape(H, D, M)[PERM].reshape(H * D, M)
    qnw_p = q_norm_w.reshape(H, D)[PERM].reshape(H * D)
    wo_tp = wo.T.reshape(H, D, M)[PERM].reshape(H * D, M)  # wo.T rows = hd

    wq_pre = np.ascontiguousarray(
        wq_p.T.reshape(8, 128, 8, 128).transpose(1, 2, 0, 3)).astype(bf16)
    wk_pre = np.ascontiguousarray(
        wk.T.reshape(8, 128, 256).transpose(1, 0, 2)).astype(bf16)
    wv_pre = np.ascontiguousarray(
        wv.T.reshape(8, 128, 256).transpose(1, 0, 2)).astype(bf16)
    wo_pre = np.ascontiguousarray(
        wo_tp.reshape(8, 128, M).transpose(1, 0, 2)).astype(bf16)
    qw_pre = np.ascontiguousarray(qnw_p.reshape(8, 128).T)
    kw_pre = np.ascontiguousarray(k_norm_w.reshape(2, 128).T)

    slopes = _alibi_slopes(H)
    # K-side aug rows: row 0 = 1.0 (pairs with -8*slope*q), rows 1,2 carry
    # the ramp 8*(kpos-512) split into pieces exactly representable in
    # bf16's 8-bit significand: 8m = 32*(m//4) + 8*(m%4).
    m = np.arange(SK, dtype=np.int64) - 512
    kaug = np.zeros((3, SK), np.float32)
    kaug[0, :] = 1.0
    kaug[1, :] = 32.0 * (m // 4).astype(np.float64)
    kaug[2, :] = 8.0 * (m % 4).astype(np.float64)
    # Q-side aug rows per slot (par, gp, j) -> head PERM[2*(4*gp+j)+par].
    # Row 0 is a per-q constant, so its bf16 rounding cancels in softmax.
    qaug = np.zeros((2, 3, 2, 4, SQ), np.float32)
    qi = np.arange(SQ, dtype=np.float64)
    for par in range(2):
        for gp in range(2):
            for j in range(4):
                h = PERM[2 * (4 * gp + j) + par]
                qaug[par, 0, gp, j, :] = -8.0 * slopes[h] * qi
                qaug[par, 1, gp, j, :] = slopes[h]
                qaug[par, 2, gp, j, :] = slopes[h]
    # row selectors
    sel = np.zeros((128, 3, 128), np.float32)
    sel[0, 0, :] = 1.0
    sel[64, 1, :] = 1.0
    sel[63, 2, :] = 1.0

    in_maps = []
    for b in range(B):
        for c in range(NCHUNK):
            q0 = c * SQ
            lo, hi = q0 - WIN, q0 + SQ
            fs = np.zeros((SK, M), np.float32)
            src_lo = max(lo, 0)
            fs[src_lo - lo:, :] = features[b, src_lo:hi, :]
            feat_pre = np.ascontiguousarray(
                fs.T.reshape(8, 128, SK).transpose(1, 0, 2)).astype(bf16)
            # ones column of V doubles as the chunk-0 halo mask: a zero
            # kills both the PV contribution and the denominator term.
            vone = np.ones((128, 8, KV, 2), np.float32)
            if c == 0:
                vone[:, 0:4, :, :] = 0.0
            in_maps.append({
                "feat": feat_pre, "wq": wq_pre, "wk": wk_pre, "wv": wv_pre,
                "wo": wo_pre, "qw": qw_pre, "kw": kw_pre,
                "kaug": kaug.astype(bf16), "qaug": qaug.astype(bf16),
                "sel": sel, "onesin": np.ones((128, 1), np.float32),
                "vone": vone.astype(bf16),
            })
    return in_maps


_NC_CACHE = {}


def kernel(features, wq, wk, wv, wo, q_norm_w, k_norm_w,
           num_heads=16, num_kv_heads=4, head_dim=64, sliding_window=512,
           **_unused):
    assert int(num_heads) == H and int(num_kv_heads) == KV
    assert int(head_dim) == D and int(sliding_window) == WIN
    from concourse.bass_utils import run_bass_kernel_spmd

    if "nc" not in _NC_CACHE:
        _NC_CACHE["nc"] = build_nc(for_sim=False)
    nc = _NC_CACHE["nc"]
    in_maps = make_in_maps(features, wq, wk, wv, wo, q_norm_w, k_norm_w)
    res = run_bass_kernel_spmd(nc, in_maps, core_ids=list(range(N_CORES)))
    outs = [r["out"] for r in res.results]
    full = np.stack(outs, axis=0).reshape(B, NCHUNK * SQ, M)
    return full.astype(np.float32)
